# revision 8
# baseline (speedup 1.0000x reference)
"""CenterVotingDecoder Trainium2 kernel.

Data-parallel over batch: 8 images -> 8 NeuronCores, identical program per
core. Convolutions run on the PE in split-precision fp32r ("fp19": 11
explicit mantissa bits): every tensor T is represented as T_hi + T_lo (host-
or engine-side 11-bit splits), and each conv tap contracts

    W_hi . (x_hi + x_lo)  [one K-packed matmul where K allows]
  + W_lo . x_hi           [second matmul]

recovering ~23 effective mantissa bits (~fp32 grade, needed because
normalize() amplifies raw-dv noise ~3000x at near-zero-norm pixels).

Phase A (16 fused strips of 20 output rows):
  conv1 3x3 128->64 (3 MMs/tap: cin fills K) -> h1 [hi(p0-63)|lo(p64-127)]
  conv2 3x3 64->64 (2 MMs/tap, K-packed hi/lo) -> h2 same layout
  conv3 3x3 64->32 per-row N=320 -> h3 [hi(p0-31)|lo(p32-63)] unpadded
  conv4 1x1 32->2 (2 MMs/chunk) -> raw dv -> HBM scratch (guard rows).

Phase B (full image): DMA raw dv back in a [64 x (5+2 halo rows)*320]
row-block layout, normalize, per-pixel vote keys (exact truncation via the
+2^23 floor trick), 27 compare+shift-accumulate ops into 3 histograms (halo
rows make every vote shift a pure free-dim offset), reductions -> centers.
"""

import sys

sys.path.insert(0, "/opt/trn_rl_repo")

import numpy as np

import concourse.bacc as bacc
import concourse.bass_isa as bass_isa
import concourse.mybir as mybir
import concourse.tile as tile

dt = mybir.dt
Alu = mybir.AluOpType
Act = mybir.ActivationFunctionType
Ax = mybir.AxisListType

H = W = 320
CIN = 128
S = 20                     # output rows per strip
NSTRIP = H // S
PW = W + 2                 # padded row width
IN_ROWS = S + 6
H1_ROWS = S + 4
H2_ROWS = S + 2
H1_FREE = H1_ROWS * PW     # 7728
H2_FREE = H2_ROWS * PW     # 7084
H3_FREE = S * W            # 6400
C1_CHUNKS = [484] * 15 + [468]   # even sizes (fp32r needs even N); sum 7728
C2_CHUNK = 506             # 14 * 506 = 7084
C2_NCH = H2_FREE // C2_CHUNK
QP = 64                    # halo-layout partitions
RB = 5                     # interior rows per halo partition
HF = (RB + 2) * W          # 2240
INT_A, INT_B = W, W + RB * W   # interior free range [320, 1920)
F23 = 8388608.0            # 2^23 for the exact floor trick
LABEL_THRESHOLD = 100.0
EPS = 1e-12

_CACHE = {}


def _split11(a):
    """Split fp32 array into hi + lo, both valid fp32r (11 explicit mantissa
    bits, RNE). a == hi + lo to ~23 bits."""
    a = np.ascontiguousarray(np.asarray(a, np.float32))

    def rnd(v):
        u = v.view(np.uint32)
        r = (u + np.uint32(0x7FF) + ((u >> np.uint32(12)) & np.uint32(1))) \
            & np.uint32(0xFFFFF000)
        return r.view(np.float32)

    hi = rnd(a)
    lo = rnd((a - hi).astype(np.float32))
    return hi, lo


def _emit_floor(nc, sb, out_ap, in_ap, shape):
    """out = floor(in), exact for in >= 0 (verified on HW)."""
    t1 = sb.tile(shape, dt.float32, tag="flr1")
    t2 = sb.tile(shape, dt.float32, tag="flr2")
    g = sb.tile(shape, dt.float32, tag="flrg")
    nc.vector.tensor_scalar(t1[:], in_ap, F23, None, op0=Alu.add)
    nc.vector.tensor_scalar(t2[:], t1[:], -F23, None, op0=Alu.add)
    nc.vector.tensor_tensor(out=g[:], in0=t2[:], in1=in_ap, op=Alu.is_gt)
    nc.vector.tensor_tensor(out=out_ap, in0=t2[:], in1=g[:], op=Alu.subtract)


def build_program():
    import concourse.bass as bass

    nc = bacc.Bacc(None, target_bir_lowering=False)

    def _zr(ap, zt):
        """Zero an fp32r AP via broadcast copy from a rounded zero tile."""
        p = ap.shape[0]
        nc.vector.tensor_copy(ap, zt[0:p, 0:1].to_broadcast(list(ap.shape)))

    xhi_d = nc.dram_tensor("xhi", [CIN, H, W], dt.float32r, kind="ExternalInput")
    xlo_d = nc.dram_tensor("xlo", [CIN, H, W], dt.float32r, kind="ExternalInput")
    lbl_d = nc.dram_tensor("lblh", [QP, HF], dt.float32, kind="ExternalInput")
    xc_d = nc.dram_tensor("xch", [QP, HF], dt.float32, kind="ExternalInput")
    yc_d = nc.dram_tensor("ych", [QP, HF], dt.float32, kind="ExternalInput")
    w1h_d = nc.dram_tensor("w1h", [CIN, 9 * 64], dt.float32r, kind="ExternalInput")
    w1l_d = nc.dram_tensor("w1l", [CIN, 9 * 64], dt.float32r, kind="ExternalInput")
    w2A_d = nc.dram_tensor("w2A", [CIN, 9 * 64], dt.float32r, kind="ExternalInput")
    w2B_d = nc.dram_tensor("w2B", [64, 9 * 64], dt.float32r, kind="ExternalInput")
    w3A_d = nc.dram_tensor("w3A", [CIN, 9 * 32], dt.float32r, kind="ExternalInput")
    w3B_d = nc.dram_tensor("w3B", [64, 9 * 32], dt.float32r, kind="ExternalInput")
    w4A_d = nc.dram_tensor("w4A", [64, 2], dt.float32r, kind="ExternalInput")
    w4B_d = nc.dram_tensor("w4B", [32, 2], dt.float32r, kind="ExternalInput")
    b1_d = nc.dram_tensor("b1", [64, 1], dt.float32, kind="ExternalInput")
    b2_d = nc.dram_tensor("b2", [64, 1], dt.float32, kind="ExternalInput")
    b3_d = nc.dram_tensor("b3", [32, 1], dt.float32, kind="ExternalInput")
    b4_d = nc.dram_tensor("b4", [2, 1], dt.float32, kind="ExternalInput")

    dv_d = nc.dram_tensor("dv", [2, H, W], dt.float32, kind="ExternalOutput")
    cent_d = nc.dram_tensor("cent", [4, 2], dt.int32, kind="ExternalOutput")

    with tile.TileContext(nc) as tc:
        with (
            tc.tile_pool(name="wpool", bufs=1) as wp,
            tc.tile_pool(name="dram", bufs=1, space="DRAM") as dp,
        ):
            w1h = wp.tile([CIN, 9 * 64], dt.float32r)
            w1l = wp.tile([CIN, 9 * 64], dt.float32r)
            w2A = wp.tile([CIN, 9 * 64], dt.float32r)
            w2B = wp.tile([64, 9 * 64], dt.float32r)
            w3A = wp.tile([CIN, 9 * 32], dt.float32r)
            w3B = wp.tile([64, 9 * 32], dt.float32r)
            w4A = wp.tile([64, 2], dt.float32r)
            w4B = wp.tile([32, 2], dt.float32r)
            for t, d in ((w1h, w1h_d), (w1l, w1l_d), (w2A, w2A_d),
                         (w2B, w2B_d), (w3A, w3A_d), (w3B, w3B_d),
                         (w4A, w4A_d), (w4B, w4B_d)):
                nc.sync.dma_start(t[:], d[:])
            b1 = wp.tile([64, 1], dt.float32)
            b2 = wp.tile([64, 1], dt.float32)
            b3 = wp.tile([32, 1], dt.float32)
            b4 = wp.tile([2, 1], dt.float32)
            for t, d in ((b1, b1_d), (b2, b2_d), (b3, b3_d), (b4, b4_d)):
                nc.sync.dma_start(t[:], d[:])

            # dv scratch with guard rows -1 and 320 (rows 0 and 321 here)
            dvr = dp.tile([2, H + 2, W], dt.float32)
            zrow = wp.tile([2, W], dt.float32)
            nc.vector.memset(zrow[:], 0.0)
            nc.sync.dma_start(dvr[:, 0, :], zrow[:])
            nc.sync.dma_start(dvr[:, H + 1, :], zrow[:])
            z32 = wp.tile([128, 1], dt.float32)
            nc.vector.memset(z32[:], 0.0)
            zt = wp.tile([128, 1], dt.float32r)
            nc.vector.tensor_copy(zt[:], z32[:])

            # ---------------- Phase A: conv pipeline ----------------
            with (
                tc.tile_pool(name="pa", bufs=1) as pa,
                tc.tile_pool(name="ptmp", bufs=3) as ptmp,
                tc.tile_pool(name="cps", bufs=6, space="PSUM") as cps,
            ):
                for s in range(NSTRIP):
                    r0 = s * S
                    in_th = pa.tile([CIN, 1 + IN_ROWS * PW + 4], dt.float32r,
                                    tag="in_th")
                    in_tl = pa.tile([CIN, 1 + IN_ROWS * PW + 4], dt.float32r,
                                    tag="in_tl")
                    first = r0 - 3
                    v0 = max(0, first)
                    v1 = min(H, r0 + S + 3)
                    skip = v0 - first
                    nv = v1 - v0
                    for t, d in ((in_th, xhi_d), (in_tl, xlo_d)):
                        dst = t[:, 1 + skip * PW: 1 + (skip + nv) * PW]
                        dst = dst.rearrange("p (r c) -> p r c", c=PW)[:, :, 1:W + 1]
                        nc.sync.dma_start(dst, d[:, v0:v1, :])
                        if skip:
                            _zr(t[:, 0:1 + skip * PW], zt)
                        if IN_ROWS - skip - nv:
                            _zr(t[:, 1 + (skip + nv) * PW: 1 + IN_ROWS * PW], zt)
                        _zr(t[:, 0:2], zt)
                        _zr(t[:, PW:PW + (IN_ROWS - 1) * PW]
                            .rearrange("p (r c) -> p r c", c=PW)[:, :, 0:2], zt)
                        _zr(t[:, 1 + IN_ROWS * PW - 1:], zt)

                    # ---- conv1: 3 MMs per tap ----
                    h1 = pa.tile([CIN, 1 + H1_FREE + 4], dt.float32r, tag="h1")
                    o = 0
                    for cn in C1_CHUNKS:
                        ps = cps.tile([64, 484], dt.float32, tag="cps")
                        for t in range(9):
                            ky, kx = divmod(t, 3)
                            q = o + ky * PW + kx
                            wsl = slice(t * 64, (t + 1) * 64)
                            nc.tensor.matmul(ps[:, :cn], w1h[:, wsl],
                                             in_th[:, q:q + cn],
                                             start=(t == 0), stop=False)
                            nc.tensor.matmul(ps[:, :cn], w1h[:, wsl],
                                             in_tl[:, q:q + cn],
                                             start=False, stop=False)
                            nc.tensor.matmul(ps[:, :cn], w1l[:, wsl],
                                             in_th[:, q:q + cn],
                                             start=False, stop=(t == 8))
                        tmp = ptmp.tile([64, 484], dt.float32, tag="tmp1")
                        nc.vector.tensor_scalar(tmp[:, :cn], ps[:, :cn], b1[:],
                                                0.0, op0=Alu.add, op1=Alu.max)
                        nc.scalar.copy(h1[0:64, 1 + o:1 + o + cn], tmp[:, :cn])
                        nc.vector.tensor_tensor(
                            out=h1[64:128, 1 + o:1 + o + cn], in0=tmp[:, :cn],
                            in1=h1[0:64, 1 + o:1 + o + cn], op=Alu.subtract)
                        o += cn
                    if s == 0:
                        _zr(h1[:, 0:1 + 2 * PW], zt)
                    if s == NSTRIP - 1:
                        _zr(h1[:, 1 + 22 * PW:], zt)
                    _zr(h1[:, 0:2], zt)
                    _zr(h1[:, PW:PW + (H1_ROWS - 1) * PW]
                        .rearrange("p (r c) -> p r c", c=PW)[:, :, 0:2], zt)
                    _zr(h1[:, 1 + H1_FREE - 1:], zt)

                    # ---- conv2: 2 MMs per tap (K-packed hi/lo) ----
                    h2 = pa.tile([CIN, 1 + H2_FREE + 4], dt.float32r, tag="h2")
                    for c in range(C2_NCH):
                        o = c * C2_CHUNK
                        ps = cps.tile([64, C2_CHUNK], dt.float32, tag="cps")
                        for t in range(9):
                            ky, kx = divmod(t, 3)
                            q = o + ky * PW + kx
                            wsl = slice(t * 64, (t + 1) * 64)
                            nc.tensor.matmul(ps[:], w2A[:, wsl],
                                             h1[:, q:q + C2_CHUNK],
                                             start=(t == 0), stop=False)
                            nc.tensor.matmul(ps[:], w2B[:, wsl],
                                             h1[0:64, q:q + C2_CHUNK],
                                             start=False, stop=(t == 8))
                        tmp = ptmp.tile([64, C2_CHUNK], dt.float32, tag="tmp2")
                        nc.vector.tensor_scalar(tmp[:], ps[:], b2[:], 0.0,
                                                op0=Alu.add, op1=Alu.max)
                        nc.scalar.copy(h2[0:64, 1 + o:1 + o + C2_CHUNK], tmp[:])
                        nc.vector.tensor_tensor(
                            out=h2[64:128, 1 + o:1 + o + C2_CHUNK], in0=tmp[:],
                            in1=h2[0:64, 1 + o:1 + o + C2_CHUNK],
                            op=Alu.subtract)
                    if s == 0:
                        _zr(h2[:, 0:1 + 1 * PW], zt)
                    if s == NSTRIP - 1:
                        _zr(h2[:, 1 + 21 * PW:], zt)
                    _zr(h2[:, 0:2], zt)
                    _zr(h2[:, PW:PW + (H2_ROWS - 1) * PW]
                        .rearrange("p (r c) -> p r c", c=PW)[:, :, 0:2], zt)
                    _zr(h2[:, 1 + H2_FREE - 1:], zt)

                    # ---- conv3: per-row, 2 MMs per tap ----
                    h3 = pa.tile([64, H3_FREE], dt.float32r, tag="h3")
                    for j in range(S):
                        ps = cps.tile([32, W], dt.float32, tag="cps")
                        for t in range(9):
                            ky, kx = divmod(t, 3)
                            q = 1 + (j + ky) * PW + kx
                            wsl = slice(t * 32, (t + 1) * 32)
                            nc.tensor.matmul(ps[:], w3A[:, wsl],
                                             h2[:, q:q + W],
                                             start=(t == 0), stop=False)
                            nc.tensor.matmul(ps[:], w3B[:, wsl],
                                             h2[0:64, q:q + W],
                                             start=False, stop=(t == 8))
                        tmp = ptmp.tile([32, W], dt.float32, tag="tmp3")
                        nc.vector.tensor_scalar(tmp[:], ps[:], b3[:], 0.0,
                                                op0=Alu.add, op1=Alu.max)
                        nc.scalar.copy(h3[0:32, j * W:(j + 1) * W], tmp[:])
                        nc.vector.tensor_tensor(
                            out=h3[32:64, j * W:(j + 1) * W], in0=tmp[:],
                            in1=h3[0:32, j * W:(j + 1) * W], op=Alu.subtract)

                    # ---- conv4 (1x1): 2 MMs per chunk -> dvr strip ----
                    off = 0
                    while off < H3_FREE:
                        n = min(512, H3_FREE - off)
                        ps4 = cps.tile([2, 512], dt.float32, tag="cps")
                        nc.tensor.matmul(ps4[:, :n], w4A[:],
                                         h3[:, off:off + n],
                                         start=True, stop=False)
                        nc.tensor.matmul(ps4[:, :n], w4B[:],
                                         h3[0:32, off:off + n],
                                         start=False, stop=True)
                        st4 = ptmp.tile([2, 512], dt.float32, tag="st4")
                        nc.scalar.activation(st4[:, :n], ps4[:, :n],
                                             Act.Identity, bias=b4[:])
                        dst = dvr[:].rearrange("c h w -> c (h w)")[
                            :, (r0 + 1) * W + off:(r0 + 1) * W + off + n]
                        nc.sync.dma_start(dst, st4[:, :n])
                        off += n

            # ---------------- Phase B: normalize + hough ----------------
            with tc.tile_pool(name="pb", bufs=1) as pb:
                dvh = pb.tile([QP, 2, HF], dt.float32)
                # dst element order (q, ch, r) -> src ch*103040 + 5q*320 + r
                dvr_flat = dvr[:].rearrange("c h w -> (c h w)")
                src_ap = bass.AP(dvr_flat.tensor, dvr_flat.offset,
                                 [[RB * W, QP], [(H + 2) * W, 2], [1, HF]])
                nc.sync.dma_start(dvh[:], src_ap)
                lblh = pb.tile([QP, HF], dt.float32)
                xch = pb.tile([QP, HF], dt.float32)
                ych = pb.tile([QP, HF], dt.float32)
                nc.sync.dma_start(lblh[:], lbl_d[:])
                nc.sync.dma_start(xch[:], xc_d[:])
                nc.sync.dma_start(ych[:], yc_d[:])

                dx = dvh[:, 0, :]
                dy = dvh[:, 1, :]
                t1 = pb.tile([QP, HF], dt.float32, tag='bigA')
                t2 = pb.tile([QP, HF], dt.float32, tag='bigB')
                nc.vector.tensor_tensor(out=t1[:], in0=dx, in1=dx, op=Alu.mult)
                nc.vector.tensor_tensor(out=t2[:], in0=dy, in1=dy, op=Alu.mult)
                nc.vector.tensor_tensor(out=t1[:], in0=t1[:], in1=t2[:],
                                        op=Alu.add)
                nc.scalar.activation(t1[:], t1[:], Act.Sqrt)
                nc.vector.tensor_scalar(t1[:], t1[:], EPS, None, op0=Alu.max)
                rec = pb.tile([QP, HF], dt.float32, tag='bigC')
                nc.vector.reciprocal(rec[:], t1[:])
                nc.vector.tensor_tensor(out=dx, in0=dx, in1=rec[:], op=Alu.mult)
                nc.vector.tensor_tensor(out=dy, in0=dy, in1=rec[:], op=Alu.mult)

                # dv output: interior rows only
                dvflat = dv_d[:].rearrange("c h w -> (c h w)")
                for ch in range(2):
                    dst_ap = bass.AP(dvflat.tensor,
                                     dvflat.offset + ch * H * W,
                                     [[RB * W, QP], [1, RB * W]])
                    nc.sync.dma_start(dst_ap, dvh[:, ch, INT_A:INT_B])

                # vote keys
                vx = pb.tile([QP, HF], dt.float32, tag='bigD')
                vy = pb.tile([QP, HF], dt.float32, tag='bigE')
                nc.vector.tensor_tensor(out=vx[:], in0=xch[:], in1=dx,
                                        op=Alu.add)
                nc.vector.tensor_scalar(vx[:], vx[:], 0.0, float(W - 1),
                                        op0=Alu.max, op1=Alu.min)
                nc.vector.tensor_tensor(out=vy[:], in0=ych[:], in1=dy,
                                        op=Alu.add)
                nc.vector.tensor_scalar(vy[:], vy[:], 0.0, float(H - 1),
                                        op0=Alu.max, op1=Alu.min)
                fvx = pb.tile([QP, HF], dt.float32, tag='bigA')
                fvy = pb.tile([QP, HF], dt.float32, tag='bigB')
                _emit_floor(nc, pb, fvx[:], vx[:], [QP, HF])
                _emit_floor(nc, pb, fvy[:], vy[:], [QP, HF])
                ix = pb.tile([QP, HF], dt.float32, tag='bigC')
                iy = pb.tile([QP, HF], dt.float32, tag='bigD')
                nc.vector.tensor_tensor(out=ix[:], in0=fvx[:], in1=xch[:],
                                        op=Alu.subtract)
                nc.vector.tensor_tensor(out=iy[:], in0=fvy[:], in1=ych[:],
                                        op=Alu.subtract)
                key = pb.tile([QP, HF], dt.float32, tag='bigE')
                nc.vector.scalar_tensor_tensor(
                    out=key[:], in0=iy[:], scalar=3.0, in1=ix[:],
                    op0=Alu.mult, op1=Alu.add)
                nc.vector.scalar_tensor_tensor(
                    out=key[:], in0=lblh[:], scalar=9.0, in1=key[:],
                    op0=Alu.mult, op1=Alu.add)

                # histograms
                hists = []
                for li in (1, 2, 3):
                    hl = pb.tile([QP, HF], dt.float16, tag=f"hist{li}")
                    nc.vector.memset(hl[:], 0.0)
                    hists.append(hl)
                for li in (1, 2, 3):
                    hl = hists[li - 1]
                    for t in range(9):
                        oy, ox = divmod(t, 3)
                        cval = float(li * 9 + oy * 3 + ox + 4)
                        d = (oy - 1) * W + (ox - 1)
                        mk = pb.tile([QP, HF], dt.float16, tag="mk")
                        nc.vector.tensor_scalar(mk[:], key[:], cval, None,
                                                op0=Alu.is_equal)
                        a = max(0, d)
                        b = HF + min(0, d)
                        nc.vector.tensor_tensor(
                            out=hl[:, a:b], in0=hl[:, a:b],
                            in1=mk[:, a - d:b - d], op=Alu.add)

                # index map f = y*512 + x
                fi = pb.tile([QP, HF], dt.float32, tag='bigC')
                nc.vector.scalar_tensor_tensor(
                    out=fi[:], in0=ych[:], scalar=512.0, in1=xch[:],
                    op0=Alu.mult, op1=Alu.add)

                cent = pb.tile([1, 8], dt.float32)
                nc.vector.memset(cent[:], -1.0)
                sm = pb.tile([QP, 8], dt.float32, tag="sm")
                for li in (1, 2, 3):
                    hl = hists[li - 1]
                    hint = hl[:, INT_A:INT_B]
                    mx = pb.tile([QP, 1], dt.float16, tag="mx")
                    nc.vector.tensor_reduce(mx[:], hint, axis=Ax.X, op=Alu.max)
                    gmx = pb.tile([QP, 1], dt.float32, tag="gmx")
                    nc.gpsimd.partition_all_reduce(
                        gmx[:], mx[:], channels=QP,
                        reduce_op=bass_isa.ReduceOp.max)
                    eq = pb.tile([QP, W * RB], dt.float32, tag='bigD')
                    nc.vector.tensor_scalar(eq[:], hint, gmx[:], None,
                                            op0=Alu.is_equal)
                    cand = pb.tile([QP, W * RB], dt.float32, tag='bigE')
                    nc.vector.scalar_tensor_tensor(
                        out=cand[:], in0=eq[:], scalar=-F23,
                        in1=fi[:, INT_A:INT_B], op0=Alu.mult, op1=Alu.add)
                    nc.vector.tensor_reduce(sm[:, 0:1], cand[:], axis=Ax.X,
                                            op=Alu.min)
                    nc.vector.tensor_scalar(sm[:, 1:2], sm[:, 0:1], -1.0, None,
                                            op0=Alu.mult)
                    nc.gpsimd.partition_all_reduce(
                        sm[:, 2:3], sm[:, 1:2], channels=QP,
                        reduce_op=bass_isa.ReduceOp.max)
                    nc.vector.tensor_scalar(sm[:, 3:4], sm[:, 2:3], -1.0, F23,
                                            op0=Alu.mult, op1=Alu.add)
                    nc.vector.tensor_scalar(sm[:, 4:5], sm[:, 3:4],
                                            1.0 / 512.0, None, op0=Alu.mult)
                    _emit_floor(nc, pb, sm[:, 5:6], sm[:, 4:5], [QP, 1])
                    nc.vector.scalar_tensor_tensor(
                        out=sm[:, 6:7], in0=sm[:, 5:6], scalar=-512.0,
                        in1=sm[:, 3:4], op0=Alu.mult, op1=Alu.add)
                    gmf = pb.tile([QP, 1], dt.float32, tag="gmf")
                    nc.vector.tensor_scalar(gmf[:], gmx[:], LABEL_THRESHOLD,
                                            None, op0=Alu.is_gt)
                    for k, col in ((6, 2 * li), (5, 2 * li + 1)):  # cx, cy
                        nc.vector.tensor_scalar(sm[:, 7:8], sm[:, k:k + 1],
                                                1.0, None, op0=Alu.add)
                        nc.vector.tensor_tensor(out=sm[:, 7:8],
                                                in0=sm[:, 7:8], in1=gmf[:],
                                                op=Alu.mult)
                        nc.vector.tensor_scalar(cent[0:1, col:col + 1],
                                                sm[0:1, 7:8], -1.0, None,
                                                op0=Alu.add)

                centi = pb.tile([1, 8], dt.int32)
                nc.vector.tensor_copy(centi[:], cent[:])
                nc.sync.dma_start(
                    cent_d[:].rearrange("a b -> (a b)"), centi[:])

    nc.compile()
    return nc


def _halo_pack(img, sentinel):
    """[320, 320] -> [64, 2240] with one halo row above/below per block."""
    pad = np.full((H + 2, W), sentinel, img.dtype)
    pad[1:H + 1] = img
    q = np.arange(QP)[:, None]
    r = np.arange(HF)[None, :]
    return np.ascontiguousarray(pad[q * RB + r // W, r % W])


def _pack_static(w1, b1, w2, b2, w3, b3, w4, b4):
    w1 = np.asarray(w1, np.float32)
    w2 = np.asarray(w2, np.float32)
    w3 = np.asarray(w3, np.float32)
    w4 = np.asarray(w4, np.float32)
    w1h, w1l = _split11(np.concatenate(
        [w1[:, :, ky, kx].T for ky in range(3) for kx in range(3)], axis=1))
    w2h, w2l = _split11(np.concatenate(
        [w2[:, :, ky, kx].T for ky in range(3) for kx in range(3)], axis=1))
    w3h, w3l = _split11(np.concatenate(
        [w3[:, :, ky, kx].T for ky in range(3) for kx in range(3)], axis=1))
    w4h, w4l = _split11(w4[:, :, 0, 0].T)
    ych, xch = np.meshgrid(np.arange(H, dtype=np.float32),
                           np.arange(W, dtype=np.float32), indexing="ij")
    static = {
        "w1h": w1h, "w1l": w1l,
        "w2A": np.ascontiguousarray(np.concatenate([w2h, w2h], axis=0)),
        "w2B": w2l,
        "w3A": np.ascontiguousarray(np.concatenate([w3h, w3h], axis=0)),
        "w3B": w3l,
        "w4A": np.ascontiguousarray(np.concatenate([w4h, w4h], axis=0)),
        "w4B": w4l,
        "b1": np.asarray(b1, np.float32).reshape(64, 1),
        "b2": np.asarray(b2, np.float32).reshape(64, 1),
        "b3": np.asarray(b3, np.float32).reshape(32, 1),
        "b4": np.asarray(b4, np.float32).reshape(2, 1),
        "xch": _halo_pack(xch, 0.0),
        "ych": _halo_pack(ych, 0.0),
    }
    return static


def make_in_maps(x, label, w1, b1, w2, b2, w3, b3, w4, b4):
    x = np.asarray(x, np.float32)
    label = np.asarray(label)
    static = _pack_static(w1, b1, w2, b2, w3, b3, w4, b4)
    in_maps = []
    for c in range(x.shape[0]):
        m = dict(static)
        xhi, xlo = _split11(x[c])
        m["xhi"] = xhi
        m["xlo"] = xlo
        m["lblh"] = _halo_pack(label[c].astype(np.float32), -1.0)
        in_maps.append(m)
    return in_maps


def get_program():
    if "nc" not in _CACHE:
        _CACHE["nc"] = build_program()
    return _CACHE["nc"]


def kernel(x, label, w1, b1, w2, b2, w3, b3, w4, b4):
    from concourse.bass_utils import run_bass_kernel_spmd

    nc = get_program()
    in_maps = make_in_maps(x, label, w1, b1, w2, b2, w3, b3, w4, b4)
    res = run_bass_kernel_spmd(nc, in_maps, core_ids=list(range(len(in_maps))))
    dv = np.stack([r["dv"] for r in res.results])
    cent = np.stack([r["cent"] for r in res.results]).astype(np.int32)
    return dv, cent


def run_timed(inputs, iters=10):
    """Chained on-device executions; returns mean per-iteration ns."""
    import time

    import jax
    import jax.numpy as jnp
    from jax.sharding import Mesh, PartitionSpec
    from jax.experimental.shard_map import shard_map

    import concourse.mybir as mybir_
    from concourse import bass2jax

    bass2jax.install_neuronx_cc_hook()
    nc = get_program()
    in_maps = make_in_maps(**inputs)
    n_cores = len(in_maps)

    partition_name = (nc.partition_id_tensor.name
                      if nc.partition_id_tensor else None)
    in_names, out_names, out_avals, zero_outs = [], [], [], []
    for alloc in nc.m.functions[0].allocations:
        if not isinstance(alloc, mybir_.MemoryLocationSet):
            continue
        name = alloc.memorylocations[0].name
        if alloc.kind == "ExternalInput":
            if name != partition_name:
                in_names.append(name)
        elif alloc.kind == "ExternalOutput":
            out_names.append(name)
            shape = tuple(alloc.tensor_shape)
            dtype = mybir_.dt.np(alloc.dtype)
            out_avals.append(jax.core.ShapedArray(shape, dtype))
            zero_outs.append(np.zeros(shape, dtype))
    n_params = len(in_names)
    n_outs = len(out_names)
    all_in = list(in_names) + list(out_names)

    def _body(*args):
        operands = list(args)
        operands.append(bass2jax.partition_id_tensor())
        outs = bass2jax._bass_exec_p.bind(
            *operands,
            out_avals=tuple(out_avals),
            in_names=tuple(all_in + [partition_name]
                           if partition_name else all_in),
            out_names=tuple(out_names),
            lowering_input_output_aliases=(),
            sim_require_finite=True,
            sim_require_nnan=True,
            nc=nc,
        )
        return tuple(outs)

    devices = jax.devices()[:n_cores]
    mesh = Mesh(np.asarray(devices), ("core",))
    in_specs = (PartitionSpec("core"),) * (n_params + n_outs)
    out_specs = (PartitionSpec("core"),) * n_outs
    donate = tuple(range(n_params, n_params + n_outs))
    fn = jax.jit(shard_map(_body, mesh=mesh, in_specs=in_specs,
                           out_specs=out_specs, check_rep=False),
                 donate_argnums=donate, keep_unused=True)

    concat_in = [jnp.asarray(np.concatenate(
        [np.asarray(in_maps[c][k]) for c in range(n_cores)], axis=0))
        for k in in_names]
    outs = tuple(jnp.asarray(np.zeros((n_cores * z.shape[0], *z.shape[1:]),
                                      z.dtype)) for z in zero_outs)
    outs = fn(*concat_in, *outs)
    outs = fn(*concat_in, *outs)
    jax.block_until_ready(outs)
    t0 = time.perf_counter()
    for _ in range(iters):
        outs = fn(*concat_in, *outs)
    jax.block_until_ready(outs)
    dt_s = (time.perf_counter() - t0) / iters
    return dt_s * 1e9
